# revision 1
# baseline (speedup 1.0000x reference)
"""Multi-head attention (B=1, L=2048, D=1024, H=16) on 8 TRN2 NeuronCores.

Sharding: tensor-parallel over heads. Core i computes heads 2i, 2i+1:
  - projections with column shards of w_q/w_k/w_v (128 cols each)
  - full attention for its 2 heads
  - partial output projection with the matching 128-row shard of w_o
Host sums the 8 partial outputs (row-split w_concat => partial-sum combine).

On-device layout (all matmuls bf16, fp32 PSUM accumulate):
  - host ships q^T/k^T/v^T so the contraction dim (D) is on SBUF partitions;
    inputs land as single 2MB DMAs (fan out across all 16 SDMA engines)
  - projections emit qh^T/kh^T [128 = 2 heads x 64, L] directly
  - scores transposed: S^T[k,q] via lhsT=kh^T slice, rhs=qh^T; heads A/B in
    PE row groups 0/64 (K=64 matmuls pack); exp on ScalarE, scale=1/8 folded
  - P~ @ V col-group packed: head A -> PSUM partitions 0-63 via
    tile_position (0,0), head B -> 64-127 via (0,64); both heads share one
    [128, 1024] accumulator, which is already the concat^T layout
  - softmax denominators: P~ tiles accumulated on VectorE (bf16), column-sum
    via a ones-vector matmul; reciprocal computed partition-parallel after an
    SBUF spread DMA; broadcast back over 64 partitions with K=1 matmuls
  - out_partial (bf16) = concat_local @ wo_shard + b_o (bias added as a K=1
    ones matmul into the same PSUM group); host sums partials in fp32
"""

import os
import numpy as np
import ml_dtypes

import concourse.bass as bass
import concourse.mybir as mybir
import concourse.tile as tile
from concourse import bacc
from concourse.bass import ts
from concourse.bass_utils import run_bass_kernel_spmd
from concourse.masks import make_identity

P = 128
L = 2048
D = 1024
DH = 64
NCORES = 8
BF16 = mybir.dt.bfloat16
F32 = mybir.dt.float32
AF = mybir.ActivationFunctionType
ALU = mybir.AluOpType

TRACE = False  # test.py flips this to get an NTFF profile / exec_time_ns
LAST_RESULT = {}

_CACHED_NC = None


def _build():
    nc = bacc.Bacc("TRN2", target_bir_lowering=False, debug=False, num_devices=NCORES)

    qT = nc.dram_tensor("qT", [P, D // P, L], BF16, kind="ExternalInput")
    kT = nc.dram_tensor("kT", [P, D // P, L], BF16, kind="ExternalInput")
    vT = nc.dram_tensor("vT", [P, D // P, L], BF16, kind="ExternalInput")
    wq = nc.dram_tensor("wq", [P, D // P, P], BF16, kind="ExternalInput")
    wk = nc.dram_tensor("wk", [P, D // P, P], BF16, kind="ExternalInput")
    wv = nc.dram_tensor("wv", [P, D // P, P], BF16, kind="ExternalInput")
    bq = nc.dram_tensor("bq", [P, 1], F32, kind="ExternalInput")
    bk = nc.dram_tensor("bk", [P, 1], F32, kind="ExternalInput")
    bv = nc.dram_tensor("bv", [P, 1], F32, kind="ExternalInput")
    wo = nc.dram_tensor("wo", [P, D], BF16, kind="ExternalInput")
    bo = nc.dram_tensor("bo", [1, D], BF16, kind="ExternalInput")
    bo_bc = nc.dram_tensor("bo_bc", [P, D], F32, kind="ExternalInput")
    out = nc.dram_tensor("out", [L, D], BF16, kind="ExternalOutput")

    KT = D // P  # 8 contraction tiles for the projections
    LT = L // P  # 16 seq tiles

    with tile.TileContext(nc) as tc:
        with (
            tc.tile_pool(name="const", bufs=1) as const_pool,
            tc.tile_pool(name="inputs", bufs=1) as in_pool,
            tc.tile_pool(name="proj", bufs=1) as proj_pool,
            tc.tile_pool(name="work", bufs=1) as work_pool,
        ):
            identity = const_pool.tile([P, P], BF16)
            make_identity(nc, identity[:])
            ones_c = const_pool.tile([P, P], BF16)
            nc.vector.memset(ones_c[:], 1.0)
            scr = const_pool.tile([1, 32], F32)
            nc.scalar.activation(scr[:], ones_c[0:1, 0:32], AF.Exp)

            # ---- stage inputs: small tensors first, then one big DMA per
            # activation tensor (q on sync, k on scalar, v on gpsimd) ----
            wq_sb = in_pool.tile([P, KT, P], BF16)
            wk_sb = in_pool.tile([P, KT, P], BF16)
            wv_sb = in_pool.tile([P, KT, P], BF16)
            nc.sync.dma_start(wq_sb[:], wq[:])
            nc.scalar.dma_start(wk_sb[:], wk[:])
            nc.gpsimd.dma_start(wv_sb[:], wv[:])
            bq_sb = in_pool.tile([P, 1], F32)
            bk_sb = in_pool.tile([P, 1], F32)
            bv_sb = in_pool.tile([P, 1], F32)
            nc.sync.dma_start(bq_sb[:], bq[:])
            nc.scalar.dma_start(bk_sb[:], bk[:])
            nc.gpsimd.dma_start(bv_sb[:], bv[:])
            wo_sb = in_pool.tile([P, D], BF16)
            nc.gpsimd.dma_start(wo_sb[:], wo[:])
            bo_sb = in_pool.tile([1, D], BF16)
            nc.gpsimd.dma_start(bo_sb[:], bo[:])
            bo_bc_sb = in_pool.tile([P, D], F32)
            nc.gpsimd.dma_start(bo_bc_sb[:], bo_bc[:])

            qT_sb = in_pool.tile([P, KT, L], BF16)
            kT_sb = in_pool.tile([P, KT, L], BF16)
            vT_sb = in_pool.tile([P, KT, L], BF16)
            for dst, src in ((qT_sb, qT), (kT_sb, kT), (vT_sb, vT)):
                nc.sync.dma_start(dst[:, 0:3, :], src[:, 0:3, :])
                nc.scalar.dma_start(dst[:, 3:6, :], src[:, 3:6, :])
                nc.gpsimd.dma_start(dst[:, 6:8, :], src[:, 6:8, :])

            # ---- projections: qh^T / kh^T / vh^T  [128 (2 heads * 64), L] ----
            qhT = proj_pool.tile([P, L], BF16)
            khT = proj_pool.tile([P, L], BF16)
            vhT = proj_pool.tile([P, L], BF16)
            with tc.tile_pool(name="pp1", bufs=2, space="PSUM") as pp1:
                for w_sb, b_sb, x_sb, dst in (
                    (wq_sb, bq_sb, qT_sb, qhT),
                    (wk_sb, bk_sb, kT_sb, khT),
                    (wv_sb, bv_sb, vT_sb, vhT),
                ):
                    for n in range(L // 512):
                        ps = pp1.tile([P, 512], F32, tag="projps")
                        for t in range(KT):
                            nc.tensor.matmul(
                                ps[:],
                                w_sb[:, t, :],
                                x_sb[:, t, ts(n, 512)],
                                start=(t == 0),
                                stop=(t == KT - 1),
                            )
                        nc.vector.tensor_scalar(
                            dst[:, ts(n, 512)], ps[:], b_sb[:], None, op0=ALU.add
                        )

                # vh natural layout [kseq, 128]: cols 0:64 head A, 64:128 head B
                vh_sb = proj_pool.tile([P, LT, P], BF16)
                for t2 in range(LT):
                    pst = pp1.tile([P, P], BF16, tag="projps")
                    nc.tensor.transpose(pst[:], vhT[:, ts(t2, P)], identity[:])
                    nc.vector.tensor_copy(vh_sb[:, t2, :], pst[:])

            # ---- attention: heads in PE row groups (S^T) / col groups (AV),
            # qseq processed in halves of 1024 ----
            lhsT_c = work_pool.tile([P, L], BF16)  # normalized concat^T
            u_all = work_pool.tile([P, L], F32)  # unnormalized concat^T
            dall = work_pool.tile([1, 2, L], F32)  # denominators (partition 0)
            dallr = work_pool.tile([1, 2, L], BF16)  # 1/denominators
            dsp = work_pool.tile([P, 32], F32)
            dspb = work_pool.tile([P, 32], BF16)
            accs = {}
            with (
                tc.tile_pool(name="att_ps", bufs=1, space="PSUM") as att_ps,
                tc.tile_pool(name="pt_pool", bufs=2) as pt_pool,
                tc.tile_pool(name="acc_pool", bufs=2) as acc_pool,
            ):
                for qh in (0, 1):
                    av = att_ps.tile([P, 1024], F32, tag="av", name=f"av_{qh}")
                    for kt in range(LT):
                        pts = {}
                        sts = {}
                        for h in (0, 1):
                            st = att_ps.tile(
                                [P, 1024], F32, tag="st", bufs=3,
                                name=f"st{h}_{qh}_{kt}",
                            )
                            sts[h] = st
                        for j in (0, 1):
                            for h in (0, 1):
                                nc.tensor.matmul(
                                    sts[h][:, ts(j, 512)],
                                    khT[ts(h, DH), ts(kt, P)],
                                    qhT[ts(h, DH), qh * 1024 + j * 512 : qh * 1024 + (j + 1) * 512],
                                )
                        for h in (0, 1):
                            pt = pt_pool.tile(
                                [P, 1024], BF16, tag=f"pt{h}", name=f"pt{h}_{qh}_{kt}"
                            )
                            nc.scalar.activation(pt[:], sts[h][:], AF.Exp, scale=0.125)
                            pts[h] = pt
                        for j in (0, 1):
                            for h in (0, 1):
                                nc.tensor.matmul(
                                    av[ts(h, DH), ts(j, 512)],
                                    vh_sb[:, kt, ts(h, DH)],
                                    pts[h][:, ts(j, 512)],
                                    start=(kt == 0),
                                    stop=(kt == LT - 1),
                                    tile_position=(0, DH * h),
                                )
                        for h in (0, 1):
                            a = accs.get((h, qh))
                            if a is None:
                                a = acc_pool.tile(
                                    [P, 1024], BF16, tag=f"acc{h}", name=f"acc{h}_{qh}"
                                )
                                accs[(h, qh)] = a
                                nc.vector.tensor_copy(a[:], pts[h][:])
                            else:
                                nc.vector.tensor_tensor(
                                    a[:], a[:], pts[h][:], op=ALU.add
                                )
                    nc.vector.tensor_copy(u_all[:, ts(qh, 1024)], av[:])

            # ---- denominators: column-sum, spread, invert, broadcast ----
            with tc.tile_pool(name="fin_ps", bufs=1, space="PSUM") as fin_ps:
                for h in (0, 1):
                    for qh in (0, 1):
                        dcs = fin_ps.tile([1, 1024], F32, tag="dcs", name=f"dcs{h}{qh}")
                        for j in (0, 1):
                            nc.tensor.matmul(
                                dcs[:, ts(j, 512)],
                                ones_c[:, 0:1],
                                accs[(h, qh)][:, ts(j, 512)],
                            )
                        nc.vector.tensor_copy(dall[0:1, h, ts(qh, 1024)], dcs[:])
                # spread 4096 denominators across partitions, invert, put back
                nc.sync.dma_start(
                    dsp[:], dall[0:1, :, :].rearrange("a h q -> a (h q)")
                )
                nc.vector.reciprocal(dsp[:], dsp[:])
                nc.vector.tensor_copy(dspb[:], dsp[:])
                nc.sync.dma_start(
                    dallr[0:1, :, :].rearrange("a h q -> a (h q)"), dspb[:]
                )
                # broadcast 1/d over 64 partitions per head; scale u -> lhsT_c
                bc = fin_ps.tile([P, L], F32, tag="bc")
                for h in (0, 1):
                    for j2 in range(L // 512):
                        nc.tensor.matmul(
                            bc[ts(h, DH), ts(j2, 512)],
                            ones_c[0:1, 0:DH],
                            dallr[0:1, h, ts(j2, 512)],
                            tile_position=(0, DH * h),
                        )
                nc.vector.tensor_tensor(lhsT_c[:], u_all[:], bc[:], op=ALU.mult)

            # ---- output projection: out_partial = concat_local @ wo + b_o ----
            with (
                tc.tile_pool(name="op_ps", bufs=4, space="PSUM") as op_ps,
                tc.tile_pool(name="out_pool", bufs=3) as out_pool,
            ):
                for m in range(LT):
                    osb = out_pool.tile([P, D], BF16, tag="osb")
                    for n in (0, 1):
                        ps = op_ps.tile([P, 512], F32, tag="ops")
                        if n == 0:
                            nc.tensor.matmul(
                                ps[:], lhsT_c[:, ts(m, P)], wo_sb[:, ts(n, 512)]
                            )
                            nc.vector.tensor_tensor(
                                osb[:, ts(n, 512)],
                                ps[:],
                                bo_bc_sb[:, ts(n, 512)],
                                op=ALU.add,
                            )
                        else:
                            nc.tensor.matmul(
                                ps[:],
                                lhsT_c[:, ts(m, P)],
                                wo_sb[:, ts(n, 512)],
                                start=True,
                                stop=False,
                            )
                            nc.tensor.matmul(
                                ps[:],
                                ones_c[0:1, :],
                                bo_sb[0:1, ts(n, 512)],
                                start=False,
                                stop=True,
                            )
                            nc.scalar.copy(osb[:, ts(n, 512)], ps[:])
                    (nc.sync if m % 2 == 0 else nc.gpsimd).dma_start(
                        out[ts(m, P), :], osb[:]
                    )

    nc.compile()
    return nc


def kernel(q, k, v, w_q, b_q, w_k, b_k, w_v, b_v, w_o, b_o):
    global _CACHED_NC, LAST_RESULT
    if _CACHED_NC is None:
        _CACHED_NC = _build()
    nc = _CACHED_NC

    bf16 = ml_dtypes.bfloat16

    def tile_T(x):  # [L, D] -> [128, D//128, L] contiguous
        xt = np.asarray(x, np.float32)[0].T  # [D, L]
        return np.ascontiguousarray(
            xt.reshape(D // P, P, L).transpose(1, 0, 2)
        ).astype(bf16)

    def tile_w(w):  # [D, 128] -> [128, D//128, 128] contiguous
        return np.ascontiguousarray(
            w.reshape(D // P, P, P).transpose(1, 0, 2)
        ).astype(bf16)

    q2 = tile_T(q)
    k2 = tile_T(k)
    v2 = tile_T(v)
    w_q = np.asarray(w_q, np.float32)
    w_k = np.asarray(w_k, np.float32)
    w_v = np.asarray(w_v, np.float32)
    w_o = np.asarray(w_o, np.float32)
    b_q = np.asarray(b_q, np.float32)
    b_k = np.asarray(b_k, np.float32)
    b_v = np.asarray(b_v, np.float32)
    b_o = np.asarray(b_o, np.float32)

    in_maps = []
    for i in range(NCORES):
        sl = slice(P * i, P * (i + 1))
        bo_i = (
            b_o.reshape(1, D).astype(bf16) if i == 0 else np.zeros((1, D), bf16)
        )
        bo_bc_i = (
            np.ascontiguousarray(np.broadcast_to(b_o, (P, D))).astype(np.float32)
            if i == 0
            else np.zeros((P, D), np.float32)
        )
        in_maps.append(
            {
                "qT": q2,
                "kT": k2,
                "vT": v2,
                "wq": tile_w(w_q[:, sl]),
                "wk": tile_w(w_k[:, sl]),
                "wv": tile_w(w_v[:, sl]),
                "bq": np.ascontiguousarray(b_q[sl]).reshape(P, 1),
                "bk": np.ascontiguousarray(b_k[sl]).reshape(P, 1),
                "bv": np.ascontiguousarray(b_v[sl]).reshape(P, 1),
                "wo": np.ascontiguousarray(w_o[sl, :]).astype(bf16),
                "bo": bo_i,
                "bo_bc": bo_bc_i,
            }
        )

    kwargs = {}
    if TRACE:
        tdir = "/tmp/bass_trace"
        os.makedirs(tdir, exist_ok=True)
        kwargs["tmpdir"] = tdir
    res = run_bass_kernel_spmd(nc, in_maps, list(range(NCORES)), trace=TRACE, **kwargs)
    LAST_RESULT = {
        "exec_time_ns": res.exec_time_ns,
        "trace_path": (res.instructions_and_trace or (None, None))[1],
    }
    acc = np.zeros((L, D), np.float64)
    for i in range(NCORES):
        acc += res.results[i]["out"].astype(np.float64)
    return acc.astype(np.float32).reshape(1, L, D)



# revision 5
# speedup vs baseline: 1.2907x; 1.2907x over previous
"""Multi-head attention (B=1, L=2048, D=1024, H=16) on 8 TRN2 NeuronCores.

Sharding: tensor-parallel over heads. Core i computes heads 2i, 2i+1:
  - projections with column shards of w_q/w_k/w_v (128 cols each)
  - full attention for its 2 heads
  - partial output projection with the matching 128-row shard of w_o
Host sums the 8 partial outputs and adds b_o once.

Fully software-pipelined single pass, built to keep PE dense (HAM-warm) and
ScalarE (exp, the per-core floor: 2*2048^2 elems at 1 elem/cycle/lane)
saturated from ~5us onward:
  - flat loop over 64 iterations: (q-quarter 0..3) x (k-tile 0..15)
  - per iter: PE scores S^T [128,1024] fp32 (2 heads row-packed), ScalarE
    exp -> pt bf16, then (lag 1) PE av (col-packed heads) + denominator
    ones-matmuls accumulated in PSUM (no VectorE reduction work at all)
  - k/v/q projections interleaved into early iterations (PE filler work),
    biases folded into the PSUM->SBUF evacuation on VectorE
  - vh (natural [k,dh] layout) produced by DMA-xbar transposes, not PE
  - per-quarter tail: reciprocal of d, K=1 fp32 broadcast matmul, normalize,
    then output projection + store, all overlapped with the next quarter
  - PSUM budget exactly 8 banks: st 2x2 + av 1 + dc 1 + shared mm 2
"""

import os
import numpy as np
import ml_dtypes

import concourse.bass as bass
import concourse.mybir as mybir
import concourse.tile as tile
from concourse import bacc
from concourse.bass import ts
from concourse.bass_utils import run_bass_kernel_spmd

P = 128
L = 2048
D = 1024
DH = 64
NCORES = 8
NQ = 4  # q quarters
QW = 512  # quarter width
KT = 16  # k tiles of 128
TQ = 8  # contraction chunks of 128 for projections
BF16 = mybir.dt.bfloat16
F32 = mybir.dt.float32
AF = mybir.ActivationFunctionType
ALU = mybir.AluOpType

TRACE = False  # test.py flips this to get an NTFF profile / exec_time_ns
LAST_RESULT = {}

_CACHED_NC = None


def _build():
    nc = bacc.Bacc("TRN2", target_bir_lowering=False, debug=False, num_devices=NCORES)

    qT = nc.dram_tensor("qT", [P, NQ, TQ, QW], BF16, kind="ExternalInput")
    kT = nc.dram_tensor("kT", [P, KT, TQ, P], BF16, kind="ExternalInput")
    vT = nc.dram_tensor("vT", [P, KT, TQ, P], BF16, kind="ExternalInput")
    wq = nc.dram_tensor("wq", [P, TQ, P], BF16, kind="ExternalInput")
    wk = nc.dram_tensor("wk", [P, TQ, P], BF16, kind="ExternalInput")
    wv = nc.dram_tensor("wv", [P, TQ, P], BF16, kind="ExternalInput")
    bq = nc.dram_tensor("bq", [P, 1], F32, kind="ExternalInput")
    bk = nc.dram_tensor("bk", [P, 1], F32, kind="ExternalInput")
    bv = nc.dram_tensor("bv", [P, 1], F32, kind="ExternalInput")
    wo = nc.dram_tensor("wo", [P, D], BF16, kind="ExternalInput")
    out = nc.dram_tensor("out", [KT, P, D], BF16, kind="ExternalOutput")

    with tile.TileContext(nc) as tc:
        with (
            tc.tile_pool(name="const", bufs=1) as const_pool,
            tc.tile_pool(name="inputs", bufs=1) as in_pool,
            tc.tile_pool(name="proj", bufs=1) as proj_pool,
            tc.tile_pool(name="work", bufs=1) as work_pool,
            tc.tile_pool(name="pt_pool", bufs=3) as pt_pool,
            tc.tile_pool(name="osb_pool", bufs=3) as osb_pool,
        ):
            ones_b = const_pool.tile([P, 1], BF16)
            nc.vector.memset(ones_b[:], 1.0)
            ones_f = const_pool.tile([1, DH], F32)
            nc.vector.memset(ones_f[:], 1.0)
            warm = const_pool.tile([1, 32], F32)
            # preload the exp table set while input DMAs stream
            nc.scalar.activation(warm[:], ones_f[0:1, 0:32], AF.Exp)

            # ---- stage inputs ----
            wq_sb = in_pool.tile([P, TQ, P], BF16)
            wk_sb = in_pool.tile([P, TQ, P], BF16)
            wv_sb = in_pool.tile([P, TQ, P], BF16)
            bq_sb = in_pool.tile([P, 1], F32)
            bk_sb = in_pool.tile([P, 1], F32)
            bv_sb = in_pool.tile([P, 1], F32)
            wo_sb = in_pool.tile([P, D], BF16)
            qT_sb = in_pool.tile([P, NQ, TQ, QW], BF16)
            kT_sb = in_pool.tile([P, KT, TQ, P], BF16)
            vT_sb = in_pool.tile([P, KT, TQ, P], BF16)

            # sync queue: weights for q/k paths + k quads + q quarters 0,2
            nc.sync.dma_start(wq_sb[:], wq[:])
            nc.sync.dma_start(bq_sb[:], bq[:])
            nc.gpsimd.dma_start(wk_sb[:], wk[:])
            nc.gpsimd.dma_start(bk_sb[:], bk[:])
            nc.sync.dma_start(kT_sb[:, 0:4], kT[:, 0:4])
            nc.gpsimd.dma_start(vT_sb[:, 0:4], vT[:, 0:4])
            nc.gpsimd.dma_start(wv_sb[:], wv[:])
            nc.gpsimd.dma_start(bv_sb[:], bv[:])
            nc.sync.dma_start(qT_sb[:, 0], qT[:, 0])
            nc.sync.dma_start(kT_sb[:, 4:8], kT[:, 4:8])
            nc.gpsimd.dma_start(vT_sb[:, 4:8], vT[:, 4:8])
            nc.sync.dma_start(kT_sb[:, 8:12], kT[:, 8:12])
            nc.gpsimd.dma_start(vT_sb[:, 8:12], vT[:, 8:12])
            nc.sync.dma_start(kT_sb[:, 12:16], kT[:, 12:16])
            nc.gpsimd.dma_start(vT_sb[:, 12:16], vT[:, 12:16])
            nc.sync.dma_start(qT_sb[:, 1], qT[:, 1])
            nc.gpsimd.dma_start(qT_sb[:, 2], qT[:, 2])
            nc.gpsimd.dma_start(qT_sb[:, 3], qT[:, 3])
            nc.gpsimd.dma_start(wo_sb[:], wo[:])

            # projection outputs (heads on partitions: h*64..h*64+63)
            qhT = proj_pool.tile([P, L], BF16)
            khT = proj_pool.tile([P, L], BF16)
            vhT = proj_pool.tile([P, L], BF16)
            vh = proj_pool.tile([P, KT, P], BF16)  # natural [k, dh] layout

            lhsT_c = work_pool.tile([P, L], BF16)  # normalized concat^T
            u_sb = work_pool.tile([P, QW], F32)
            dcrA = work_pool.tile([1, QW], F32)
            dcrB = work_pool.tile([1, QW], F32)

            with (
                tc.tile_pool(name="st_ps", bufs=2, space="PSUM") as st_ps,
                tc.tile_pool(name="av_ps", bufs=1, space="PSUM") as av_ps,
                tc.tile_pool(name="dc_ps", bufs=1, space="PSUM") as dc_ps,
                tc.tile_pool(name="mm_ps", bufs=2, space="PSUM") as mm_ps,
            ):

                def qproj(qi):
                    ps = mm_ps.tile([P, QW], F32, tag="mm", name=f"qp{qi}")
                    for t in range(TQ):
                        nc.tensor.matmul(
                            ps[:],
                            wq_sb[:, t, :],
                            qT_sb[:, qi, t, :],
                            start=(t == 0),
                            stop=(t == TQ - 1),
                        )
                    nc.vector.tensor_scalar(
                        qhT[:, ts(qi, QW)], ps[:], bq_sb[:], None, op0=ALU.add
                    )

                def kproj(g):
                    ps = mm_ps.tile([P, QW], F32, tag="mm", name=f"kp{g}")
                    for t in range(TQ):
                        nc.tensor.matmul(
                            ps[:],
                            wk_sb[:, t, :],
                            kT_sb[:, ts(g, 4), t, :],
                            start=(t == 0),
                            stop=(t == TQ - 1),
                        )
                    nc.vector.tensor_scalar(
                        khT[:, ts(g, QW)], ps[:], bk_sb[:], None, op0=ALU.add
                    )

                def vproj(g):
                    ps = mm_ps.tile([P, QW], F32, tag="mm", name=f"vp{g}")
                    for t in range(TQ):
                        nc.tensor.matmul(
                            ps[:],
                            wv_sb[:, t, :],
                            vT_sb[:, ts(g, 4), t, :],
                            start=(t == 0),
                            stop=(t == TQ - 1),
                        )
                    nc.vector.tensor_scalar(
                        vhT[:, ts(g, QW)], ps[:], bv_sb[:], None, op0=ALU.add
                    )
                    # natural layout via DMA xbar transpose: [dh, k] -> [k, dh]
                    for j in range(4):
                        kt = 4 * g + j
                        nc.sync.dma_start_transpose(
                            vh[:, kt, :], vhT[:, ts(kt, P)]
                        )

                # ---- head: first quads so the pipeline can start ----
                qproj(0)
                kproj(0)
                vproj(0)

                st_tiles = {}
                pt_tiles = {}
                av_t = None
                dc_t = None

                def do_st_exp(it):
                    qi, kt = divmod(it, KT)
                    st_t = st_ps.tile([P, 2 * QW], F32, tag="st", name=f"st{it}")
                    for h in (0, 1):
                        nc.tensor.matmul(
                            st_t[:, ts(h, QW)],
                            khT[ts(h, DH), ts(kt, P)],
                            qhT[ts(h, DH), ts(qi, QW)],
                        )
                    pt_t = pt_pool.tile([P, 2 * QW], BF16, tag="pt", name=f"pt{it}")
                    nc.scalar.activation(pt_t[:], st_t[:], AF.Exp, scale=0.125)
                    st_tiles[it] = st_t
                    pt_tiles[it] = pt_t

                def do_av_dc(it):
                    nonlocal av_t, dc_t
                    qi, kt = divmod(it, KT)
                    if kt == 0:
                        av_t = av_ps.tile([P, QW], F32, tag="av", name=f"av{qi}")
                        dc_t = dc_ps.tile([33, QW], F32, tag="dc", name=f"dc{qi}")
                    pt_t = pt_tiles.pop(it)
                    first = kt == 0
                    last = kt == KT - 1
                    for h in (0, 1):
                        nc.tensor.matmul(
                            av_t[ts(h, DH), :],
                            vh[:, kt, ts(h, DH)],
                            pt_t[:, ts(h, QW)],
                            start=first,
                            stop=last,
                            tile_position=(0, DH * h),
                        )
                    nc.tensor.matmul(
                        dc_t[0:1, :],
                        ones_b[:, 0:1],
                        pt_t[:, 0:QW],
                        start=first,
                        stop=last,
                        tile_position=(0, 0),
                    )
                    nc.tensor.matmul(
                        dc_t[32:33, :],
                        ones_b[:, 0:1],
                        pt_t[:, QW : 2 * QW],
                        start=first,
                        stop=last,
                        tile_position=(0, 32),
                    )
                    st_tiles.pop(it)

                def quarter_tail(qi):
                    # d -> 1/d -> broadcast over head partition groups -> norm
                    nc.vector.tensor_copy(u_sb[:], av_t[:])
                    nc.vector.reciprocal(dcrA[:], dc_t[0:1, :])
                    nc.vector.reciprocal(dcrB[:], dc_t[32:33, :])
                    bc_t = mm_ps.tile([P, QW], F32, tag="mm", name=f"bc{qi}")
                    nc.tensor.matmul(
                        bc_t[0:DH, :], ones_f[0:1, :], dcrA[:], tile_position=(0, 0)
                    )
                    nc.tensor.matmul(
                        bc_t[DH:P, :], ones_f[0:1, :], dcrB[:], tile_position=(0, DH)
                    )
                    nc.vector.tensor_tensor(
                        lhsT_c[:, ts(qi, QW)], u_sb[:], bc_t[:], op=ALU.mult
                    )

                def outproj(m):
                    osb = osb_pool.tile([P, D], BF16, tag="osb", name=f"osb{m}")
                    for n in (0, 1):
                        ps = mm_ps.tile([P, QW], F32, tag="mm", name=f"op{m}_{n}")
                        nc.tensor.matmul(
                            ps[:], lhsT_c[:, ts(m, P)], wo_sb[:, ts(n, QW)]
                        )
                        nc.vector.tensor_copy(osb[:, ts(n, QW)], ps[:])
                    (nc.sync if m % 2 == 0 else nc.gpsimd).dma_start(
                        out[m], osb[:]
                    )

                # ---- main pipelined loop ----
                for it in range(64 + 1):
                    qi, kt = divmod(it, KT)
                    if it < 64:
                        do_st_exp(it)
                    # projection filler, scheduled just-in-time for quarter 0
                    if it == 1:
                        kproj(1)
                    elif it == 3:
                        vproj(1)
                    elif it == 5:
                        kproj(2)
                    elif it == 7:
                        vproj(2)
                    elif it == 9:
                        kproj(3)
                    elif it == 11:
                        vproj(3)
                    elif kt == 13 and qi < 3:
                        qproj(qi + 1)
                    if it > 0:
                        do_av_dc(it - 1)
                    if it >= KT and kt == 0:
                        quarter_tail(qi - 1)
                    # spread output projection of the previous quarter
                    if it > KT and 1 <= kt <= 4:
                        outproj((qi - 1) * 4 + kt - 1)
                # last quarter's output projection (tail ran at it=64)
                for m in range(12, 16):
                    outproj(m)

    nc.compile()
    return nc


def kernel(q, k, v, w_q, b_q, w_k, b_k, w_v, b_v, w_o, b_o):
    global _CACHED_NC, LAST_RESULT
    if _CACHED_NC is None:
        _CACHED_NC = _build()
    nc = _CACHED_NC

    bf16 = ml_dtypes.bfloat16

    qTf = np.ascontiguousarray(np.asarray(q, np.float32)[0].T)  # [D, L]
    kTf = np.ascontiguousarray(np.asarray(k, np.float32)[0].T)
    vTf = np.ascontiguousarray(np.asarray(v, np.float32)[0].T)
    q2 = np.ascontiguousarray(
        qTf.reshape(TQ, P, NQ, QW).transpose(1, 2, 0, 3)
    ).astype(bf16)
    k2 = np.ascontiguousarray(
        kTf.reshape(TQ, P, KT, P).transpose(1, 2, 0, 3)
    ).astype(bf16)
    v2 = np.ascontiguousarray(
        vTf.reshape(TQ, P, KT, P).transpose(1, 2, 0, 3)
    ).astype(bf16)

    w_q = np.asarray(w_q, np.float32)
    w_k = np.asarray(w_k, np.float32)
    w_v = np.asarray(w_v, np.float32)
    w_o = np.asarray(w_o, np.float32)
    b_q = np.asarray(b_q, np.float32)
    b_k = np.asarray(b_k, np.float32)
    b_v = np.asarray(b_v, np.float32)
    b_o = np.asarray(b_o, np.float32)

    def tile_w(w):  # [D, 128] -> [128, D//128, 128] contiguous
        return np.ascontiguousarray(
            w.reshape(TQ, P, P).transpose(1, 0, 2)
        ).astype(bf16)

    in_maps = []
    for i in range(NCORES):
        sl = slice(P * i, P * (i + 1))
        in_maps.append(
            {
                "qT": q2,
                "kT": k2,
                "vT": v2,
                "wq": tile_w(w_q[:, sl]),
                "wk": tile_w(w_k[:, sl]),
                "wv": tile_w(w_v[:, sl]),
                "bq": np.ascontiguousarray(b_q[sl]).reshape(P, 1),
                "bk": np.ascontiguousarray(b_k[sl]).reshape(P, 1),
                "bv": np.ascontiguousarray(b_v[sl]).reshape(P, 1),
                "wo": np.ascontiguousarray(w_o[sl, :]).astype(bf16),
            }
        )

    kwargs = {}
    if TRACE:
        import shutil

        tdir = "/tmp/bass_trace"
        shutil.rmtree(tdir, ignore_errors=True)
        os.makedirs(tdir, exist_ok=True)
        kwargs["tmpdir"] = tdir
    res = run_bass_kernel_spmd(nc, in_maps, list(range(NCORES)), trace=TRACE, **kwargs)
    LAST_RESULT = {
        "exec_time_ns": res.exec_time_ns,
        "trace_path": (res.instructions_and_trace or (None, None))[1],
    }
    acc = np.zeros((L, D), np.float64)
    for i in range(NCORES):
        acc += res.results[i]["out"].reshape(L, D).astype(np.float64)
    acc += b_o.astype(np.float64)
    return acc.astype(np.float32).reshape(1, L, D)


# revision 13
# speedup vs baseline: 1.5510x; 1.2017x over previous
"""Multi-head attention (B=1, L=2048, D=1024, H=16) on 8 TRN2 NeuronCores.

Sharding: tensor-parallel over heads. Core i computes heads 2i, 2i+1:
  - projections with column shards of w_q/w_k/w_v (128 cols each)
  - full attention for its 2 heads
  - partial output projection with the matching 128-row shard of w_o
Host sums the 8 partial outputs and adds b_o once.

Fully software-pipelined single pass, built to keep PE dense (HAM-warm) and
ScalarE (exp, the per-core floor: 2*2048^2 elems at 1 elem/cycle/lane)
saturated from ~5us onward:
  - flat loop over 64 iterations: (q-quarter 0..3) x (k-tile 0..15)
  - per iter: PE scores S^T [128,1024] fp32 (2 heads row-packed), ScalarE
    exp -> pt bf16, then (lag 1) PE av (col-packed heads) + denominator
    ones-matmuls accumulated in PSUM (no VectorE reduction work at all)
  - k/v/q projections interleaved into early iterations (PE filler work),
    biases folded into the PSUM->SBUF evacuation on VectorE
  - vh (natural [k,dh] layout) produced by DMA-xbar transposes, not PE
  - per-quarter tail: reciprocal of d, K=1 fp32 broadcast matmul, normalize,
    then output projection + store, all overlapped with the next quarter
  - PSUM budget exactly 8 banks: st 2x2 + av 1 + dc 1 + shared mm 2
"""

import os
import numpy as np
import ml_dtypes

import concourse.bass as bass
import concourse.mybir as mybir
import concourse.tile as tile
from concourse import bacc
from concourse.bass import ts
from concourse.bass_utils import run_bass_kernel_spmd

P = 128
L = 2048
D = 1024
DH = 64
NCORES = 8
NQ = 4  # q quarters
QW = 512  # quarter width
KT = 16  # k tiles of 128
TQ = 8  # contraction chunks of 128 for projections
BF16 = mybir.dt.bfloat16
F32 = mybir.dt.float32
AF = mybir.ActivationFunctionType
ALU = mybir.AluOpType

TRACE = False  # test.py flips this to get an NTFF profile / exec_time_ns
LAST_RESULT = {}

_CACHED_NC = None


def _build():
    nc = bacc.Bacc("TRN2", target_bir_lowering=False, debug=False, num_devices=NCORES)

    qT = nc.dram_tensor("qT", [P, NQ, TQ, QW], BF16, kind="ExternalInput")
    kT = nc.dram_tensor("kT", [P, KT, TQ, P], BF16, kind="ExternalInput")
    vT = nc.dram_tensor("vT", [P, KT, TQ, P], BF16, kind="ExternalInput")
    wq = nc.dram_tensor("wq", [P, TQ, P], BF16, kind="ExternalInput")
    wk = nc.dram_tensor("wk", [P, TQ, P], BF16, kind="ExternalInput")
    wv = nc.dram_tensor("wv", [P, TQ, P], BF16, kind="ExternalInput")
    bq = nc.dram_tensor("bq", [P, 1], F32, kind="ExternalInput")
    bk = nc.dram_tensor("bk", [P, 1], F32, kind="ExternalInput")
    bv = nc.dram_tensor("bv", [P, 1], F32, kind="ExternalInput")
    wo = nc.dram_tensor("wo", [P, D], BF16, kind="ExternalInput")
    out = nc.dram_tensor("out", [KT, P, D], BF16, kind="ExternalOutput")

    with tile.TileContext(nc) as tc:
        with (
            tc.tile_pool(name="const", bufs=1) as const_pool,
            tc.tile_pool(name="inputs", bufs=1) as in_pool,
            tc.tile_pool(name="proj", bufs=1) as proj_pool,
            tc.tile_pool(name="work", bufs=1) as work_pool,
            tc.tile_pool(name="pt_pool", bufs=3) as pt_pool,
            tc.tile_pool(name="osb_pool", bufs=3) as osb_pool,
        ):
            ones_b = const_pool.tile([P, 1], BF16)
            nc.vector.memset(ones_b[:], 1.0)
            ones_f = const_pool.tile([1, DH], F32)
            nc.vector.memset(ones_f[:], 1.0)
            warm = const_pool.tile([1, 32], F32)
            # preload the exp table set while input DMAs stream
            nc.scalar.activation(warm[:], ones_f[0:1, 0:32], AF.Exp)

            # ---- stage inputs ----
            wq_sb = in_pool.tile([P, TQ, P], BF16)
            wk_sb = in_pool.tile([P, TQ, P], BF16)
            wv_sb = in_pool.tile([P, TQ, P], BF16)
            bq_sb = in_pool.tile([P, 1], F32)
            bk_sb = in_pool.tile([P, 1], F32)
            bv_sb = in_pool.tile([P, 1], F32)
            wo_sb = in_pool.tile([P, D], BF16)
            qT_sb = in_pool.tile([P, NQ, TQ, QW], BF16)
            kT_sb = in_pool.tile([P, KT, TQ, P], BF16)
            vT_sb = in_pool.tile([P, KT, TQ, P], BF16)

            # sync queue: q-proj critical path first, then k; gpsimd: v path
            nc.sync.dma_start(wq_sb[:], wq[:])
            nc.sync.dma_start(bq_sb[:], bq[:])
            nc.sync.dma_start(qT_sb[:, 0], qT[:, 0])
            nc.gpsimd.dma_start(wk_sb[:], wk[:])
            nc.gpsimd.dma_start(bk_sb[:], bk[:])
            nc.gpsimd.dma_start(wv_sb[:], wv[:])
            nc.gpsimd.dma_start(bv_sb[:], bv[:])
            nc.sync.dma_start(kT_sb[:, 0:2], kT[:, 0:2])
            nc.gpsimd.dma_start(vT_sb[:, 0:2], vT[:, 0:2])
            nc.sync.dma_start(kT_sb[:, 2:4], kT[:, 2:4])
            nc.gpsimd.dma_start(vT_sb[:, 2:4], vT[:, 2:4])
            nc.sync.dma_start(kT_sb[:, 4:8], kT[:, 4:8])
            nc.gpsimd.dma_start(vT_sb[:, 4:8], vT[:, 4:8])
            nc.sync.dma_start(kT_sb[:, 8:12], kT[:, 8:12])
            nc.gpsimd.dma_start(vT_sb[:, 8:12], vT[:, 8:12])
            nc.sync.dma_start(kT_sb[:, 12:16], kT[:, 12:16])
            nc.gpsimd.dma_start(vT_sb[:, 12:16], vT[:, 12:16])
            nc.sync.dma_start(qT_sb[:, 1], qT[:, 1])
            nc.gpsimd.dma_start(qT_sb[:, 2], qT[:, 2])
            nc.sync.dma_start(qT_sb[:, 3], qT[:, 3])
            nc.gpsimd.dma_start(wo_sb[:], wo[:])

            # projection outputs (heads on partitions: h*64..h*64+63)
            qhT = proj_pool.tile([P, L], BF16)
            khT = proj_pool.tile([P, L], BF16)
            vhT = proj_pool.tile([P, L], BF16)
            vh = proj_pool.tile([P, KT, P], BF16)  # natural [k, dh] layout

            lhsT_c = work_pool.tile([P, L], BF16)  # normalized concat^T
            u_sb = work_pool.tile([P, QW], F32)
            dsb = work_pool.tile([1, 2 * QW], F32)  # d rows gathered
            dsp = work_pool.tile([DH, 2 * QW // DH], F32)  # spread for recip
            dcr = work_pool.tile([1, 2 * QW], F32)  # 1/d back in row layout

            with (
                tc.tile_pool(name="st_ps", bufs=2, space="PSUM") as st_ps,
                tc.tile_pool(name="av_ps", bufs=1, space="PSUM") as av_ps,
                tc.tile_pool(name="dc_ps", bufs=1, space="PSUM") as dc_ps,
                tc.tile_pool(name="mm_ps", bufs=2, space="PSUM") as mm_ps,
            ):

                def qproj(qi):
                    ps = mm_ps.tile([P, QW], F32, tag="mm", name=f"qp{qi}")
                    for t in range(TQ):
                        nc.tensor.matmul(
                            ps[:],
                            wq_sb[:, t, :],
                            qT_sb[:, qi, t, :],
                            start=(t == 0),
                            stop=(t == TQ - 1),
                        )
                    nc.vector.tensor_scalar(
                        qhT[:, ts(qi, QW)], ps[:], bq_sb[:], None, op0=ALU.add
                    )

                def kproj(g):
                    ps = mm_ps.tile([P, QW], F32, tag="mm", name=f"kp{g}")
                    for t in range(TQ):
                        nc.tensor.matmul(
                            ps[:],
                            wk_sb[:, t, :],
                            kT_sb[:, ts(g, 4), t, :],
                            start=(t == 0),
                            stop=(t == TQ - 1),
                        )
                    nc.vector.tensor_scalar(
                        khT[:, ts(g, QW)], ps[:], bk_sb[:], None, op0=ALU.add
                    )

                def vproj(g):
                    ps = mm_ps.tile([P, QW], F32, tag="mm", name=f"vp{g}")
                    for t in range(TQ):
                        nc.tensor.matmul(
                            ps[:],
                            wv_sb[:, t, :],
                            vT_sb[:, ts(g, 4), t, :],
                            start=(t == 0),
                            stop=(t == TQ - 1),
                        )
                    nc.vector.tensor_scalar(
                        vhT[:, ts(g, QW)], ps[:], bv_sb[:], None, op0=ALU.add
                    )
                    # natural layout via DMA xbar transpose: [dh, k] -> [k, dh]
                    for j in range(4):
                        kt = 4 * g + j
                        nc.sync.dma_start_transpose(
                            vh[:, kt, :], vhT[:, ts(kt, P)]
                        )

                # ---- head: first quads so the pipeline can start ----
                qproj(0)
                kproj(0)
                vproj(0)

                st_tiles = {}
                pt_tiles = {}
                av_t = None
                dc_t = None

                def do_st_exp(it):
                    qi, kt = divmod(it, KT)
                    st_t = st_ps.tile([P, 2 * QW], F32, tag="st", name=f"st{it}")
                    for h in (0, 1):
                        nc.tensor.matmul(
                            st_t[:, ts(h, QW)],
                            khT[ts(h, DH), ts(kt, P)],
                            qhT[ts(h, DH), ts(qi, QW)],
                        )
                    pt_t = pt_pool.tile([P, 2 * QW], BF16, tag="pt", name=f"pt{it}")
                    nc.scalar.activation(pt_t[:], st_t[:], AF.Exp, scale=0.125)
                    st_tiles[it] = st_t
                    pt_tiles[it] = pt_t

                def do_av_dc(it):
                    nonlocal av_t, dc_t
                    qi, kt = divmod(it, KT)
                    if kt == 0:
                        av_t = av_ps.tile([P, QW], F32, tag="av", name=f"av{qi}")
                        dc_t = dc_ps.tile([33, QW], F32, tag="dc", name=f"dc{qi}")
                    pt_t = pt_tiles.pop(it)
                    first = kt == 0
                    last = kt == KT - 1
                    for h in (0, 1):
                        nc.tensor.matmul(
                            av_t[ts(h, DH), :],
                            vh[:, kt, ts(h, DH)],
                            pt_t[:, ts(h, QW)],
                            start=first,
                            stop=last,
                            tile_position=(0, DH * h),
                        )
                    nc.tensor.matmul(
                        dc_t[0:1, :],
                        ones_b[:, 0:1],
                        pt_t[:, 0:QW],
                        start=first,
                        stop=last,
                        tile_position=(0, 0),
                    )
                    nc.tensor.matmul(
                        dc_t[32:33, :],
                        ones_b[:, 0:1],
                        pt_t[:, QW : 2 * QW],
                        start=first,
                        stop=last,
                        tile_position=(0, 32),
                    )
                    st_tiles.pop(it)

                def quarter_tail(qi):
                    # d -> 1/d (reciprocal on a partition-spread copy: DVE
                    # reciprocal cost is ~6.5ns/elem of free dim) -> broadcast
                    # over head partition groups -> normalize
                    nc.vector.tensor_copy(u_sb[:], av_t[:])
                    nc.vector.tensor_copy(dsb[0:1, 0:QW], dc_t[0:1, :])
                    nc.vector.tensor_copy(dsb[0:1, QW : 2 * QW], dc_t[32:33, :])
                    nc.sync.dma_start(dsp[:], dsb[:])
                    nc.vector.reciprocal(dsp[:], dsp[:])
                    nc.sync.dma_start(dcr[:], dsp[:])
                    bc_t = mm_ps.tile([P, QW], F32, tag="mm", name=f"bc{qi}")
                    nc.tensor.matmul(
                        bc_t[0:DH, :],
                        ones_f[0:1, :],
                        dcr[0:1, 0:QW],
                        tile_position=(0, 0),
                    )
                    nc.tensor.matmul(
                        bc_t[DH:P, :],
                        ones_f[0:1, :],
                        dcr[0:1, QW : 2 * QW],
                        tile_position=(0, DH),
                    )
                    nc.vector.tensor_tensor(
                        lhsT_c[:, ts(qi, QW)], u_sb[:], bc_t[:], op=ALU.mult
                    )

                def outproj(m):
                    osb = osb_pool.tile([P, D], BF16, tag="osb", name=f"osb{m}")
                    for n in (0, 1):
                        ps = mm_ps.tile([P, QW], F32, tag="mm", name=f"op{m}_{n}")
                        nc.tensor.matmul(
                            ps[:], lhsT_c[:, ts(m, P)], wo_sb[:, ts(n, QW)]
                        )
                        nc.vector.tensor_copy(osb[:, ts(n, QW)], ps[:])
                    nc.gpsimd.dma_start(out[m], osb[:])

                # ---- main pipelined loop ----
                for it in range(64 + 1):
                    qi, kt = divmod(it, KT)
                    if it < 64:
                        do_st_exp(it)
                    # projection filler, scheduled just-in-time for quarter 0
                    if it == 1:
                        kproj(1)
                    elif it == 3:
                        vproj(1)
                    elif it == 5:
                        kproj(2)
                    elif it == 7:
                        vproj(2)
                    elif it == 9:
                        kproj(3)
                    elif it == 11:
                        vproj(3)
                    elif kt == 13 and qi < 3:
                        qproj(qi + 1)
                    if it > 0:
                        do_av_dc(it - 1)
                    if it >= KT and kt == 0:
                        quarter_tail(qi - 1)
                    # spread output projection of the previous quarter
                    if it > KT and kt in (4, 6, 8, 10):
                        outproj((qi - 1) * 4 + (kt - 4) // 2)
                # last quarter's output projection (tail ran at it=64)
                for m in range(12, 16):
                    outproj(m)

    nc.compile()
    return nc


def kernel(q, k, v, w_q, b_q, w_k, b_k, w_v, b_v, w_o, b_o):
    global _CACHED_NC, LAST_RESULT
    if _CACHED_NC is None:
        _CACHED_NC = _build()
    nc = _CACHED_NC

    bf16 = ml_dtypes.bfloat16

    qTf = np.ascontiguousarray(np.asarray(q, np.float32)[0].T)  # [D, L]
    kTf = np.ascontiguousarray(np.asarray(k, np.float32)[0].T)
    vTf = np.ascontiguousarray(np.asarray(v, np.float32)[0].T)
    q2 = np.ascontiguousarray(
        qTf.reshape(TQ, P, NQ, QW).transpose(1, 2, 0, 3)
    ).astype(bf16)
    k2 = np.ascontiguousarray(
        kTf.reshape(TQ, P, KT, P).transpose(1, 2, 0, 3)
    ).astype(bf16)
    v2 = np.ascontiguousarray(
        vTf.reshape(TQ, P, KT, P).transpose(1, 2, 0, 3)
    ).astype(bf16)

    w_q = np.asarray(w_q, np.float32)
    w_k = np.asarray(w_k, np.float32)
    w_v = np.asarray(w_v, np.float32)
    w_o = np.asarray(w_o, np.float32)
    b_q = np.asarray(b_q, np.float32)
    b_k = np.asarray(b_k, np.float32)
    b_v = np.asarray(b_v, np.float32)
    b_o = np.asarray(b_o, np.float32)

    def tile_w(w):  # [D, 128] -> [128, D//128, 128] contiguous
        return np.ascontiguousarray(
            w.reshape(TQ, P, P).transpose(1, 0, 2)
        ).astype(bf16)

    in_maps = []
    for i in range(NCORES):
        sl = slice(P * i, P * (i + 1))
        in_maps.append(
            {
                "qT": q2,
                "kT": k2,
                "vT": v2,
                "wq": tile_w(w_q[:, sl]),
                "wk": tile_w(w_k[:, sl]),
                "wv": tile_w(w_v[:, sl]),
                "bq": np.ascontiguousarray(b_q[sl]).reshape(P, 1),
                "bk": np.ascontiguousarray(b_k[sl]).reshape(P, 1),
                "bv": np.ascontiguousarray(b_v[sl]).reshape(P, 1),
                "wo": np.ascontiguousarray(w_o[sl, :]).astype(bf16),
            }
        )

    kwargs = {}
    if TRACE:
        import shutil

        tdir = "/tmp/bass_trace"
        shutil.rmtree(tdir, ignore_errors=True)
        os.makedirs(tdir, exist_ok=True)
        kwargs["tmpdir"] = tdir
    res = run_bass_kernel_spmd(nc, in_maps, list(range(NCORES)), trace=TRACE, **kwargs)
    LAST_RESULT = {
        "exec_time_ns": res.exec_time_ns,
        "trace_path": (res.instructions_and_trace or (None, None))[1],
    }
    acc = np.zeros((L, D), np.float64)
    for i in range(NCORES):
        acc += res.results[i]["out"].reshape(L, D).astype(np.float64)
    acc += b_o.astype(np.float64)
    return acc.astype(np.float32).reshape(1, L, D)


# revision 15
# speedup vs baseline: 1.6585x; 1.0693x over previous
"""Multi-head attention (B=1, L=2048, D=1024, H=16) on 8 TRN2 NeuronCores.

Sharding: tensor-parallel over heads. Core i computes heads 2i, 2i+1:
  - projections with column shards of w_q/w_k/w_v (128 cols each)
  - full attention for its 2 heads
  - partial output projection with the matching 128-row shard of w_o
Host sums the 8 partial outputs and adds b_o once.

Fully software-pipelined single pass, built to keep PE dense (HAM-warm) and
ScalarE (exp, the per-core floor: 2*2048^2 elems at 1 elem/cycle/lane)
saturated from ~5us onward:
  - flat loop over 64 iterations: (q-quarter 0..3) x (k-tile 0..15)
  - per iter: PE scores S^T [128,1024] fp32 (2 heads row-packed), ScalarE
    exp -> pt bf16, then (lag 1) PE av (col-packed heads) + denominator
    ones-matmuls accumulated in PSUM (no VectorE reduction work at all)
  - k/v/q projections interleaved into early iterations (PE filler work),
    biases folded into the PSUM->SBUF evacuation on VectorE
  - vh (natural [k,dh] layout) produced by DMA-xbar transposes, not PE
  - per-quarter tail: reciprocal of d, K=1 fp32 broadcast matmul, normalize,
    then output projection + store, all overlapped with the next quarter
  - PSUM budget exactly 8 banks: st 2x2 + av 1 + dc 1 + shared mm 2
"""

import os
import numpy as np
import ml_dtypes

import concourse.bass as bass
import concourse.mybir as mybir
import concourse.tile as tile
from concourse import bacc
from concourse.bass import ts
from concourse.bass_utils import run_bass_kernel_spmd

P = 128
L = 2048
D = 1024
DH = 64
NCORES = 8
NQ = 4  # q quarters
QW = 512  # quarter width
KT = 16  # k tiles of 128
TQ = 8  # contraction chunks of 128 for projections
BF16 = mybir.dt.bfloat16
F32 = mybir.dt.float32
AF = mybir.ActivationFunctionType
ALU = mybir.AluOpType

TRACE = False  # test.py flips this to get an NTFF profile / exec_time_ns
LAST_RESULT = {}

_CACHED_NC = None


def _build():
    nc = bacc.Bacc("TRN2", target_bir_lowering=False, debug=False, num_devices=NCORES)

    qT = nc.dram_tensor("qT", [P, NQ, TQ, QW], BF16, kind="ExternalInput")
    kT = nc.dram_tensor("kT", [P, KT, TQ, P], BF16, kind="ExternalInput")
    vT = nc.dram_tensor("vT", [P, KT, TQ, P], BF16, kind="ExternalInput")
    wq = nc.dram_tensor("wq", [P, TQ, P], BF16, kind="ExternalInput")
    wk = nc.dram_tensor("wk", [P, TQ, P], BF16, kind="ExternalInput")
    wv = nc.dram_tensor("wv", [P, TQ, P], BF16, kind="ExternalInput")
    bq = nc.dram_tensor("bq", [P, 1], F32, kind="ExternalInput")
    bk = nc.dram_tensor("bk", [P, 1], F32, kind="ExternalInput")
    bv = nc.dram_tensor("bv", [P, 1], F32, kind="ExternalInput")
    wo = nc.dram_tensor("wo", [P, D], BF16, kind="ExternalInput")
    out = nc.dram_tensor("out", [KT, P, D], BF16, kind="ExternalOutput")

    with tile.TileContext(nc) as tc:
        with (
            tc.tile_pool(name="const", bufs=1) as const_pool,
            tc.tile_pool(name="inputs", bufs=1) as in_pool,
            tc.tile_pool(name="proj", bufs=1) as proj_pool,
            tc.tile_pool(name="work", bufs=1) as work_pool,
            tc.tile_pool(name="pt_pool", bufs=3) as pt_pool,
            tc.tile_pool(name="osb_pool", bufs=3) as osb_pool,
        ):
            ones_b = const_pool.tile([P, 1], BF16)
            nc.vector.memset(ones_b[:], 1.0)
            ones_f = const_pool.tile([1, DH], F32)
            nc.vector.memset(ones_f[:], 1.0)
            warm = const_pool.tile([1, 32], F32)
            # preload the exp table set while input DMAs stream
            nc.scalar.activation(warm[:], ones_f[0:1, 0:32], AF.Exp)

            # ---- stage inputs ----
            wq_sb = in_pool.tile([P, TQ, P], BF16)
            wk_sb = in_pool.tile([P, TQ, P], BF16)
            wv_sb = in_pool.tile([P, TQ, P], BF16)
            bq_sb = in_pool.tile([P, 1], F32)
            bk_sb = in_pool.tile([P, 1], F32)
            bv_sb = in_pool.tile([P, 1], F32)
            wo_sb = in_pool.tile([P, D], BF16)
            qT_sb = in_pool.tile([P, NQ, TQ, QW], BF16)
            kT_sb = in_pool.tile([P, KT, TQ, P], BF16)
            vT_sb = in_pool.tile([P, KT, TQ, P], BF16)

            # bulk input on the two HWDGE rings: sync = q/k path, scalar = v
            # path (scalar's queue is idle until the first exp anyway; these
            # triggers all precede the exps in its FIFO). gpsimd/SWDGE is an
            # order of magnitude slower — keep bulk data off it.
            nc.sync.dma_start(wq_sb[:], wq[:])
            nc.sync.dma_start(bq_sb[:], bq[:])
            nc.scalar.dma_start(wk_sb[:], wk[:])
            nc.scalar.dma_start(bk_sb[:], bk[:])
            nc.sync.dma_start(qT_sb[:, 0], qT[:, 0])
            nc.scalar.dma_start(wv_sb[:], wv[:])
            nc.scalar.dma_start(bv_sb[:], bv[:])
            nc.sync.dma_start(kT_sb[:, 0:2], kT[:, 0:2])
            nc.scalar.dma_start(vT_sb[:, 0:2], vT[:, 0:2])
            nc.sync.dma_start(kT_sb[:, 2:4], kT[:, 2:4])
            nc.scalar.dma_start(vT_sb[:, 2:4], vT[:, 2:4])
            nc.sync.dma_start(kT_sb[:, 4:8], kT[:, 4:8])
            nc.scalar.dma_start(vT_sb[:, 4:8], vT[:, 4:8])
            nc.sync.dma_start(kT_sb[:, 8:12], kT[:, 8:12])
            nc.scalar.dma_start(vT_sb[:, 8:12], vT[:, 8:12])
            nc.sync.dma_start(kT_sb[:, 12:16], kT[:, 12:16])
            nc.scalar.dma_start(vT_sb[:, 12:16], vT[:, 12:16])
            nc.sync.dma_start(qT_sb[:, 1], qT[:, 1])
            nc.scalar.dma_start(qT_sb[:, 2], qT[:, 2])
            nc.sync.dma_start(qT_sb[:, 3], qT[:, 3])
            nc.scalar.dma_start(wo_sb[:], wo[:])

            # projection outputs (heads on partitions: h*64..h*64+63)
            qhT = proj_pool.tile([P, L], BF16)
            khT = proj_pool.tile([P, L], BF16)
            vhT = proj_pool.tile([P, L], BF16)
            vh = proj_pool.tile([P, KT, P], BF16)  # natural [k, dh] layout

            lhsT_c = work_pool.tile([P, L], BF16)  # normalized concat^T
            u_sb = work_pool.tile([P, QW], F32)
            dsb = work_pool.tile([1, 2 * QW], F32)  # d rows gathered
            dsp = work_pool.tile([DH, 2 * QW // DH], F32)  # spread for recip
            dcr = work_pool.tile([1, 2 * QW], F32)  # 1/d back in row layout

            with (
                tc.tile_pool(name="st_ps", bufs=2, space="PSUM") as st_ps,
                tc.tile_pool(name="av_ps", bufs=1, space="PSUM") as av_ps,
                tc.tile_pool(name="dc_ps", bufs=1, space="PSUM") as dc_ps,
                tc.tile_pool(name="mm_ps", bufs=2, space="PSUM") as mm_ps,
            ):

                def qproj(qi):
                    ps = mm_ps.tile([P, QW], F32, tag="mm", name=f"qp{qi}")
                    for t in range(TQ):
                        nc.tensor.matmul(
                            ps[:],
                            wq_sb[:, t, :],
                            qT_sb[:, qi, t, :],
                            start=(t == 0),
                            stop=(t == TQ - 1),
                        )
                    nc.vector.tensor_scalar(
                        qhT[:, ts(qi, QW)], ps[:], bq_sb[:], None, op0=ALU.add
                    )

                def kproj(g):
                    ps = mm_ps.tile([P, QW], F32, tag="mm", name=f"kp{g}")
                    for t in range(TQ):
                        nc.tensor.matmul(
                            ps[:],
                            wk_sb[:, t, :],
                            kT_sb[:, ts(g, 4), t, :],
                            start=(t == 0),
                            stop=(t == TQ - 1),
                        )
                    nc.vector.tensor_scalar(
                        khT[:, ts(g, QW)], ps[:], bk_sb[:], None, op0=ALU.add
                    )

                def vproj(g):
                    ps = mm_ps.tile([P, QW], F32, tag="mm", name=f"vp{g}")
                    for t in range(TQ):
                        nc.tensor.matmul(
                            ps[:],
                            wv_sb[:, t, :],
                            vT_sb[:, ts(g, 4), t, :],
                            start=(t == 0),
                            stop=(t == TQ - 1),
                        )
                    nc.vector.tensor_scalar(
                        vhT[:, ts(g, QW)], ps[:], bv_sb[:], None, op0=ALU.add
                    )
                    # natural layout via DMA xbar transpose: [dh, k] -> [k, dh]
                    for j in range(4):
                        kt = 4 * g + j
                        nc.sync.dma_start_transpose(
                            vh[:, kt, :], vhT[:, ts(kt, P)]
                        )

                # ---- head: first quads so the pipeline can start ----
                qproj(0)
                kproj(0)
                vproj(0)

                st_tiles = {}
                pt_tiles = {}
                av_t = None
                dc_t = None

                def do_st_exp(it):
                    qi, kt = divmod(it, KT)
                    st_t = st_ps.tile([P, 2 * QW], F32, tag="st", name=f"st{it}")
                    for h in (0, 1):
                        nc.tensor.matmul(
                            st_t[:, ts(h, QW)],
                            khT[ts(h, DH), ts(kt, P)],
                            qhT[ts(h, DH), ts(qi, QW)],
                        )
                    pt_t = pt_pool.tile([P, 2 * QW], BF16, tag="pt", name=f"pt{it}")
                    nc.scalar.activation(pt_t[:], st_t[:], AF.Exp, scale=0.125)
                    st_tiles[it] = st_t
                    pt_tiles[it] = pt_t

                def do_av_dc(it):
                    nonlocal av_t, dc_t
                    qi, kt = divmod(it, KT)
                    if kt == 0:
                        av_t = av_ps.tile([P, QW], F32, tag="av", name=f"av{qi}")
                        dc_t = dc_ps.tile([33, QW], F32, tag="dc", name=f"dc{qi}")
                    pt_t = pt_tiles.pop(it)
                    first = kt == 0
                    last = kt == KT - 1
                    for h in (0, 1):
                        nc.tensor.matmul(
                            av_t[ts(h, DH), :],
                            vh[:, kt, ts(h, DH)],
                            pt_t[:, ts(h, QW)],
                            start=first,
                            stop=last,
                            tile_position=(0, DH * h),
                        )
                    nc.tensor.matmul(
                        dc_t[0:1, :],
                        ones_b[:, 0:1],
                        pt_t[:, 0:QW],
                        start=first,
                        stop=last,
                        tile_position=(0, 0),
                    )
                    nc.tensor.matmul(
                        dc_t[32:33, :],
                        ones_b[:, 0:1],
                        pt_t[:, QW : 2 * QW],
                        start=first,
                        stop=last,
                        tile_position=(0, 32),
                    )
                    st_tiles.pop(it)

                def quarter_tail(qi):
                    # d -> 1/d (reciprocal on a partition-spread copy: DVE
                    # reciprocal cost is ~6.5ns/elem of free dim) -> broadcast
                    # over head partition groups -> normalize
                    nc.vector.tensor_copy(u_sb[:], av_t[:])
                    nc.vector.tensor_copy(dsb[0:1, 0:QW], dc_t[0:1, :])
                    nc.vector.tensor_copy(dsb[0:1, QW : 2 * QW], dc_t[32:33, :])
                    nc.sync.dma_start(dsp[:], dsb[:])
                    nc.vector.reciprocal(dsp[:], dsp[:])
                    nc.sync.dma_start(dcr[:], dsp[:])
                    bc_t = mm_ps.tile([P, QW], F32, tag="mm", name=f"bc{qi}")
                    nc.tensor.matmul(
                        bc_t[0:DH, :],
                        ones_f[0:1, :],
                        dcr[0:1, 0:QW],
                        tile_position=(0, 0),
                    )
                    nc.tensor.matmul(
                        bc_t[DH:P, :],
                        ones_f[0:1, :],
                        dcr[0:1, QW : 2 * QW],
                        tile_position=(0, DH),
                    )
                    nc.vector.tensor_tensor(
                        lhsT_c[:, ts(qi, QW)], u_sb[:], bc_t[:], op=ALU.mult
                    )

                def outproj(m):
                    osb = osb_pool.tile([P, D], BF16, tag="osb", name=f"osb{m}")
                    for n in (0, 1):
                        ps = mm_ps.tile([P, QW], F32, tag="mm", name=f"op{m}_{n}")
                        nc.tensor.matmul(
                            ps[:], lhsT_c[:, ts(m, P)], wo_sb[:, ts(n, QW)]
                        )
                        nc.vector.tensor_copy(osb[:, ts(n, QW)], ps[:])
                    nc.sync.dma_start(out[m], osb[:])

                # ---- main pipelined loop ----
                for it in range(64 + 1):
                    qi, kt = divmod(it, KT)
                    if it < 64:
                        do_st_exp(it)
                    # projection filler, scheduled just-in-time for quarter 0
                    if it == 1:
                        kproj(1)
                    elif it == 3:
                        vproj(1)
                    elif it == 5:
                        kproj(2)
                    elif it == 7:
                        vproj(2)
                    elif it == 9:
                        kproj(3)
                    elif it == 11:
                        vproj(3)
                    elif kt == 13 and qi < 3:
                        qproj(qi + 1)
                    if it > 0:
                        do_av_dc(it - 1)
                    if it >= KT and kt == 0:
                        quarter_tail(qi - 1)
                    # spread output projection of the previous quarter
                    if it > KT and kt in (4, 6, 8, 10):
                        outproj((qi - 1) * 4 + (kt - 4) // 2)
                # last quarter's output projection (tail ran at it=64)
                for m in range(12, 16):
                    outproj(m)

    nc.compile()
    return nc


def kernel(q, k, v, w_q, b_q, w_k, b_k, w_v, b_v, w_o, b_o):
    global _CACHED_NC, LAST_RESULT
    if _CACHED_NC is None:
        _CACHED_NC = _build()
    nc = _CACHED_NC

    bf16 = ml_dtypes.bfloat16

    qTf = np.ascontiguousarray(np.asarray(q, np.float32)[0].T)  # [D, L]
    kTf = np.ascontiguousarray(np.asarray(k, np.float32)[0].T)
    vTf = np.ascontiguousarray(np.asarray(v, np.float32)[0].T)
    q2 = np.ascontiguousarray(
        qTf.reshape(TQ, P, NQ, QW).transpose(1, 2, 0, 3)
    ).astype(bf16)
    k2 = np.ascontiguousarray(
        kTf.reshape(TQ, P, KT, P).transpose(1, 2, 0, 3)
    ).astype(bf16)
    v2 = np.ascontiguousarray(
        vTf.reshape(TQ, P, KT, P).transpose(1, 2, 0, 3)
    ).astype(bf16)

    w_q = np.asarray(w_q, np.float32)
    w_k = np.asarray(w_k, np.float32)
    w_v = np.asarray(w_v, np.float32)
    w_o = np.asarray(w_o, np.float32)
    b_q = np.asarray(b_q, np.float32)
    b_k = np.asarray(b_k, np.float32)
    b_v = np.asarray(b_v, np.float32)
    b_o = np.asarray(b_o, np.float32)

    def tile_w(w):  # [D, 128] -> [128, D//128, 128] contiguous
        return np.ascontiguousarray(
            w.reshape(TQ, P, P).transpose(1, 0, 2)
        ).astype(bf16)

    in_maps = []
    for i in range(NCORES):
        sl = slice(P * i, P * (i + 1))
        in_maps.append(
            {
                "qT": q2,
                "kT": k2,
                "vT": v2,
                "wq": tile_w(w_q[:, sl]),
                "wk": tile_w(w_k[:, sl]),
                "wv": tile_w(w_v[:, sl]),
                "bq": np.ascontiguousarray(b_q[sl]).reshape(P, 1),
                "bk": np.ascontiguousarray(b_k[sl]).reshape(P, 1),
                "bv": np.ascontiguousarray(b_v[sl]).reshape(P, 1),
                "wo": np.ascontiguousarray(w_o[sl, :]).astype(bf16),
            }
        )

    kwargs = {}
    if TRACE:
        import shutil

        tdir = "/tmp/bass_trace"
        shutil.rmtree(tdir, ignore_errors=True)
        os.makedirs(tdir, exist_ok=True)
        kwargs["tmpdir"] = tdir
    res = run_bass_kernel_spmd(nc, in_maps, list(range(NCORES)), trace=TRACE, **kwargs)
    LAST_RESULT = {
        "exec_time_ns": res.exec_time_ns,
        "trace_path": (res.instructions_and_trace or (None, None))[1],
    }
    acc = np.zeros((L, D), np.float64)
    for i in range(NCORES):
        acc += res.results[i]["out"].reshape(L, D).astype(np.float64)
    acc += b_o.astype(np.float64)
    return acc.astype(np.float32).reshape(1, L, D)
